# revision 1
# baseline (speedup 1.0000x reference)
"""ConvNeXt MLP + parallel MoE-LoRA kernel for TRN2 (8-core data parallel).

Per-core computation (tokens sharded across cores, feature-on-partition layout):
  orig = GELU(x @ W1 + b1) @ W2 + b2                       (base MLP)
  prob_i = sum_k where(topk_idx==i, topk_probs, 0)          (routing weights)
  h_i    = GELU(x @ w_down_i)                               (LoRA down, all experts)
  moe    = sum_i (h_i * prob_i) @ w_up_i                    (weighted up-proj)
  out    = orig + moe

All GEMMs keep features on the partition dim, tokens on the free dim, so the
host ships x transposed ([DIM, NT] per core) and reassembles the transposed
output.  The expert-weighted combine folds into the final PSUM accumulation:
GEMM2's 12 K-tiles plus the stacked LoRA up-projection (K=24) accumulate into
the same PSUM bank.
"""

import numpy as np
import ml_dtypes

import concourse.bass as bass
import concourse.mybir as mybir
import concourse.tile as tile
from concourse import bacc
from concourse.masks import make_identity

F32 = mybir.dt.float32
F32R = mybir.dt.float32r
BF16 = mybir.dt.bfloat16

DIM, HID, E, R = 384, 1536, 3, 8
KT = DIM // 128    # 3  k-tiles of x / W1 rows
MT = HID // 128    # 12 hid tiles
ER = E * R         # 24 stacked lora dims
CH = 512           # token chunk (free dim per matmul)


def build_nc(NT, mm="fp32r", nrep=1, num_devices=8, act="gelu"):
    """Build the bass program for one core's shard of NT tokens.

    mm: "fp32r" (fp32 storage, full-speed replicated matmul) or "bf16".
    nrep: repeat the main compute loop (for wall-clock differencing timing).
    """
    assert NT % 128 == 0
    NCH = NT // 128                      # 128-token chunks for prob stage
    assert NCH <= 128
    wdt = F32R if mm == "fp32r" else BF16
    actf = (mybir.ActivationFunctionType.Gelu if act == "gelu"
            else mybir.ActivationFunctionType.Identity)

    def cast(ap):
        return ap

    nc = bacc.Bacc("TRN2", target_bir_lowering=False, debug=False,
                   num_devices=num_devices)

    xT = nc.dram_tensor("xT", [DIM, NT], wdt, kind="ExternalInput").ap()
    W1 = nc.dram_tensor("W1", [DIM, HID], wdt, kind="ExternalInput").ap()
    W2 = nc.dram_tensor("W2", [HID, DIM], wdt, kind="ExternalInput").ap()
    WDN = nc.dram_tensor("wdn", [DIM, ER], wdt, kind="ExternalInput").ap()
    WUP = nc.dram_tensor("wup", [ER, DIM], wdt, kind="ExternalInput").ap()
    B1 = nc.dram_tensor("b1c", [128, MT], F32, kind="ExternalInput").ap()
    B2 = nc.dram_tensor("b2c", [128, KT], F32, kind="ExternalInput").ap()
    IDX = nc.dram_tensor("idxf", [128, NCH * 2], F32, kind="ExternalInput").ap()
    PRB = nc.dram_tensor("prbf", [128, NCH * 2], F32, kind="ExternalInput").ap()
    EMAT = nc.dram_tensor("emat", [E, ER], wdt, kind="ExternalInput").ap()
    OUT = nc.dram_tensor("outT", [DIM, NT], F32, kind="ExternalOutput").ap()

    # token chunks of the main loop: CH-wide plus a remainder
    chunks = []
    off = 0
    while off < NT:
        w = min(CH, NT - off)
        chunks.append((off, w))
        off += w

    with tile.TileContext(nc) as tc:
        with (
            tc.tile_pool(name="const", bufs=1) as const,
            tc.tile_pool(name="xin", bufs=3) as xin,
            tc.tile_pool(name="hact", bufs=2) as hact,
            tc.tile_pool(name="lora", bufs=2) as lora,
            tc.tile_pool(name="outp", bufs=3) as outp,
            tc.tile_pool(name="ph", bufs=3, space="PSUM") as ph,
            tc.tile_pool(name="po", bufs=3, space="PSUM") as po,
            tc.tile_pool(name="ps", bufs=1, space="PSUM") as ps,
        ):
            ptr = po
            w1sb = const.tile([128, KT, HID], wdt)
            w2sb = const.tile([128, MT, DIM], wdt)
            wdnsb = const.tile([128, KT, ER], wdt)
            wupsb = const.tile([ER, DIM], wdt)
            b1sb = const.tile([128, MT], F32)
            b2sb = const.tile([128, KT], F32)
            idxsb = const.tile([128, NCH * 2], F32)
            prbsb = const.tile([128, NCH * 2], F32)
            ident = const.tile([128, 128], F32)
            esb = const.tile([E, ER], wdt)
            probT = const.tile([E, NT], wdt)

            make_identity(nc, ident)

            def load_small():
                # everything the first chunk + routing needs except W2
                nc.sync.dma_start(out=idxsb, in_=IDX)
                nc.sync.dma_start(out=prbsb, in_=PRB)
                nc.sync.dma_start(out=esb, in_=EMAT)
                for k in range(KT):
                    nc.sync.dma_start(out=wdnsb[:, k, :],
                                      in_=WDN[k * 128:(k + 1) * 128, :])
                nc.sync.dma_start(out=wupsb, in_=WUP)
                nc.sync.dma_start(out=b1sb, in_=B1)
                nc.sync.dma_start(out=b2sb, in_=B2)
                for k in range(KT):
                    nc.sync.dma_start(out=w1sb[:, k, :],
                                      in_=W1[k * 128:(k + 1) * 128, :])

            def load_w2():
                # needed only by the first stage2, one chunk later
                for k in range(MT):
                    nc.sync.dma_start(out=w2sb[:, k, :],
                                      in_=W2[k * 128:(k + 1) * 128, :])

            def routing():
                # probT[i, t] = sum_k where(topk_idx[t,k]==i, topk_probs[t,k], 0)
                for i in range(E):
                    eq = lora.tile([128, NCH, 2], F32, tag="eq")
                    nc.vector.tensor_scalar(
                        out=eq, in0=idxsb.rearrange("p (c k) -> p c k", k=2),
                        scalar1=float(i), scalar2=None,
                        op0=mybir.AluOpType.is_equal)
                    msk = lora.tile([128, NCH, 2], F32, tag="msk")
                    nc.vector.tensor_tensor(
                        out=msk, in0=eq,
                        in1=prbsb.rearrange("p (c k) -> p c k", k=2),
                        op=mybir.AluOpType.mult)
                    pri = lora.tile([128, NCH, 1], F32, tag="pri")
                    nc.vector.tensor_tensor(
                        out=pri, in0=msk[:, :, 0:1], in1=msk[:, :, 1:2],
                        op=mybir.AluOpType.add)
                    pri = pri[:, :, 0]
                    # transpose [128, NCH] -> [NCH, 128], linearize into probT
                    prt = ptr.tile([NCH, 128], F32, tag="po")
                    nc.tensor.transpose(prt, pri, ident)
                    stg = lora.tile([NCH, 128], wdt, tag="stg")
                    nc.vector.tensor_copy(out=stg, in_=prt)
                    nc.sync.dma_start(out=probT[i:i + 1, :], in_=stg)

            def stage1(off, w):
                """load x chunk, GEMM1+GELU, lora down + routing weight."""
                csl = slice(off, off + w)
                xsb = xin.tile([128, KT, CH], wdt, tag="x")
                for k in range(KT):
                    nc.sync.dma_start(out=xsb[:, k, :w],
                                      in_=xT[k * 128:(k + 1) * 128, csl])
                hsb = hact.tile([128, MT, CH], wdt, tag="h")
                for m in range(MT):
                    pst = ph.tile([128, CH], F32, tag="ph")
                    for k in range(KT):
                        nc.tensor.matmul(
                            pst[:, :w],
                            cast(w1sb[:, k, m * 128:(m + 1) * 128]),
                            cast(xsb[:, k, :w]),
                            start=(k == 0), stop=(k == KT - 1))
                    nc.scalar.activation(
                        out=hsb[:, m, :w], in_=pst[:, :w], func=actf,
                        bias=b1sb[:, m:m + 1], scale=1.0)
                psl = ps.tile([ER, CH], F32, tag="pl")
                for k in range(KT):
                    nc.tensor.matmul(
                        psl[:, :w], cast(wdnsb[:, k, :]), cast(xsb[:, k, :w]),
                        start=(k == 0), stop=(k == KT - 1))
                psp = ps.tile([ER, CH], F32, tag="pp")
                nc.tensor.matmul(psp[:, :w], esb, probT[:, csl],
                                 start=True, stop=True)
                hl = lora.tile([ER, CH], F32, tag="hl")
                nc.scalar.activation(out=hl[:, :w], in_=psl[:, :w], func=actf)
                hw = lora.tile([ER, CH], wdt, tag="hw")
                nc.vector.tensor_tensor(out=hw[:, :w], in0=hl[:, :w],
                                        in1=psp[:, :w], op=mybir.AluOpType.mult)
                return hsb, hw

            def stage2(off, w, hsb, hw):
                """GEMM2 + accumulated lora up, bias, store."""
                csl = slice(off, off + w)
                osb = outp.tile([128, KT, CH], F32, tag="o")
                for d in range(KT):
                    pso = po.tile([128, CH], F32, tag="po")
                    for k in range(MT):
                        nc.tensor.matmul(
                            pso[:, :w],
                            cast(w2sb[:, k, d * 128:(d + 1) * 128]),
                            cast(hsb[:, k, :w]),
                            start=(k == 0), stop=False)
                    nc.tensor.matmul(
                        pso[:, :w], cast(wupsb[:, d * 128:(d + 1) * 128]),
                        cast(hw[:, :w]), start=False, stop=True)
                    nc.vector.tensor_scalar(
                        out=osb[:, d, :w], in0=pso[:, :w],
                        scalar1=b2sb[:, d:d + 1], scalar2=None,
                        op0=mybir.AluOpType.add)
                    nc.sync.dma_start(out=OUT[d * 128:(d + 1) * 128, csl],
                                      in_=osb[:, d, :w])

            def body(_iv=None):
                load_small()
                routing()
                # software pipeline: stage2 of chunk j runs one chunk behind
                prev = None
                for ci, (off, w) in enumerate(chunks):
                    cur = (off, w) + stage1(off, w)
                    if ci == 0:
                        load_w2()
                    if prev is not None:
                        stage2(*prev)
                    prev = cur
                stage2(*prev)

            if nrep == 1:
                body()
            else:
                with tc.For_i(0, nrep, 1,
                              hint_engines=(mybir.EngineType.PE,
                                            mybir.EngineType.Activation,
                                            mybir.EngineType.DVE,
                                            mybir.EngineType.SP)):
                    body()

    nc.compile()
    return nc


# ---------------- host-side helpers ----------------

def shard_inputs(x, topk_probs, topk_idx, w_down, w_up, W1, b1, W2, b2,
                 n_cores=8, mm="fp32r", scaling=1.0):
    """Full inputs -> list of per-core in_maps (plus NT per core)."""
    npdt = np.float32 if mm == "fp32r" else ml_dtypes.bfloat16
    x_flat = np.asarray(x, np.float32).reshape(-1, DIM)
    N = x_flat.shape[0]
    assert N % (n_cores * 128) == 0
    NT = N // n_cores
    NCH = NT // 128

    W1h = np.ascontiguousarray(np.asarray(W1, np.float32)).astype(npdt)
    W2h = np.ascontiguousarray(np.asarray(W2, np.float32)).astype(npdt)
    wdn = np.concatenate([np.asarray(w_down[i], np.float32) for i in range(E)],
                         axis=1).astype(npdt)                       # [DIM, ER]
    wup = (np.concatenate([np.asarray(w_up[i], np.float32) for i in range(E)],
                          axis=0) * scaling).astype(npdt)           # [ER, DIM]
    b1c = np.ascontiguousarray(np.asarray(b1, np.float32).reshape(MT, 128).T)
    b2c = np.ascontiguousarray(np.asarray(b2, np.float32).reshape(KT, 128).T)

    idx_f = np.asarray(topk_idx).astype(np.float32)
    prb_f = np.asarray(topk_probs).astype(np.float32)

    in_maps = []
    for c in range(n_cores):
        sl = slice(c * NT, (c + 1) * NT)
        xTc = np.ascontiguousarray(x_flat[sl].T).astype(npdt)
        idxc = np.ascontiguousarray(
            idx_f[sl].reshape(NCH, 128, 2).transpose(1, 0, 2).reshape(128, NCH * 2))
        prbc = np.ascontiguousarray(
            prb_f[sl].reshape(NCH, 128, 2).transpose(1, 0, 2).reshape(128, NCH * 2))
        emat = np.zeros((E, ER), npdt)
        for i in range(E):
            emat[i, i * R:(i + 1) * R] = 1.0
        in_maps.append({
            "xT": xTc, "W1": W1h, "W2": W2h, "wdn": wdn, "wup": wup,
            "b1c": b1c, "b2c": b2c, "idxf": idxc, "prbf": prbc, "emat": emat,
        })
    return in_maps, NT


def unshard_output(results, x_shape):
    outs = [r["outT"] for r in results]          # each [DIM, NT] f32
    full = np.concatenate(outs, axis=1)          # [DIM, N]
    return np.ascontiguousarray(full.T).reshape(x_shape)


# ---------------- self-contained entry point ----------------

_NC_CACHE = {}


def _get_nc(NT, mm="fp32r", nrep=1):
    key = (NT, mm, nrep)
    if key not in _NC_CACHE:
        _NC_CACHE[key] = build_nc(NT, mm=mm, nrep=nrep, num_devices=8,
                                  act="gelu")
    return _NC_CACHE[key]


def kernel(x, gate, topk_probs, topk_idx, w_down, w_up, W1, b1, W2, b2):
    """Full (unsharded) inputs -> full output, 8-core data parallel over
    tokens.  `gate` is unused (the reference never reads it)."""
    from concourse.bass_utils import run_bass_kernel_spmd

    x = np.asarray(x)
    in_maps, NT = shard_inputs(
        x, np.asarray(topk_probs), np.asarray(topk_idx), np.asarray(w_down),
        np.asarray(w_up), np.asarray(W1), np.asarray(b1), np.asarray(W2),
        np.asarray(b2), n_cores=8, mm="fp32r", scaling=8.0 / 8.0)
    nc = _get_nc(NT, mm="fp32r", nrep=1)
    res = run_bass_kernel_spmd(nc, in_maps, core_ids=list(range(8)))
    return unshard_output(res.results, x.shape).astype(np.float32)



# revision 2
# speedup vs baseline: 1.0452x; 1.0452x over previous
"""ConvNeXt MLP + parallel MoE-LoRA kernel for TRN2 (8-core data parallel).

Per-core computation (tokens sharded across cores, feature-on-partition layout):
  orig = GELU(x @ W1 + b1) @ W2 + b2                       (base MLP)
  h    = GELU(x @ w_down_all)                               (LoRA down, stacked)
  moe  = sum_i (h_i * prob_i) @ w_up_i                      (weighted up-proj)
  out  = orig + moe

All GEMMs keep features on the partition dim, tokens on the free dim; the host
ships x transposed ([DIM, NT] per core) and reassembles the transposed output.
The per-token routing weights are expanded on the host to a [ER, NT] map
(prob of the owning expert replicated across its R lora dims), so the device
does no routing work: the weighted combine is one elementwise multiply, and
the LoRA up-projection accumulates into GEMM2's PSUM banks.
"""

import numpy as np
import ml_dtypes

import concourse.bass as bass
import concourse.mybir as mybir
import concourse.tile as tile
from concourse import bacc

F32 = mybir.dt.float32
F32R = mybir.dt.float32r
BF16 = mybir.dt.bfloat16

DIM, HID, E, R = 384, 1536, 3, 8
KT = DIM // 128    # 3  k-tiles of x / W1 rows
MT = HID // 128    # 12 hid tiles
ER = E * R         # 24 stacked lora dims
CH = 512           # token chunk (free dim per matmul)


def build_nc(NT, mm="bf16", nrep=1, num_devices=8, act="gelu"):
    """Build the bass program for one core's shard of NT tokens."""
    assert NT % 128 == 0
    wdt = F32R if mm == "fp32r" else BF16
    actf = (mybir.ActivationFunctionType.Gelu if act == "gelu"
            else mybir.ActivationFunctionType.Identity)

    nc = bacc.Bacc("TRN2", target_bir_lowering=False, debug=False,
                   num_devices=num_devices)

    xT = nc.dram_tensor("xT", [DIM, NT], wdt, kind="ExternalInput").ap()
    W1 = nc.dram_tensor("W1", [DIM, HID], wdt, kind="ExternalInput").ap()
    W2 = nc.dram_tensor("W2", [HID, DIM], wdt, kind="ExternalInput").ap()
    WDN = nc.dram_tensor("wdn", [DIM, ER], wdt, kind="ExternalInput").ap()
    WUP = nc.dram_tensor("wup", [ER, DIM], wdt, kind="ExternalInput").ap()
    B1 = nc.dram_tensor("b1c", [128, MT], F32, kind="ExternalInput").ap()
    B2 = nc.dram_tensor("b2c", [128, KT], F32, kind="ExternalInput").ap()
    PRX = nc.dram_tensor("prx", [ER, NT], wdt, kind="ExternalInput").ap()
    OUT = nc.dram_tensor("outT", [DIM, NT], F32, kind="ExternalOutput").ap()

    # token chunks of the main loop: CH-wide plus a remainder
    chunks = []
    off = 0
    while off < NT:
        w = min(CH, NT - off)
        chunks.append((off, w))
        off += w

    with tile.TileContext(nc) as tc:
        with (
            tc.tile_pool(name="const", bufs=1) as const,
            tc.tile_pool(name="xin", bufs=3) as xin,
            tc.tile_pool(name="hact", bufs=2) as hact,
            tc.tile_pool(name="lora", bufs=2) as lora,
            tc.tile_pool(name="outp", bufs=3) as outp,
            tc.tile_pool(name="ph", bufs=3, space="PSUM") as ph,
            tc.tile_pool(name="po", bufs=3, space="PSUM") as po,
            tc.tile_pool(name="ps", bufs=2, space="PSUM") as ps,
        ):
            w1sb = const.tile([128, KT, HID], wdt)
            w2sb = const.tile([128, MT, DIM], wdt)
            wdnsb = const.tile([128, KT, ER], wdt)
            wupsb = const.tile([ER, DIM], wdt)
            b1sb = const.tile([128, MT], F32)
            b2sb = const.tile([128, KT], F32)
            prxsb = const.tile([ER, NT], wdt)

            def load_x(off, w):
                csl = slice(off, off + w)
                xsb = xin.tile([128, KT, CH], wdt, tag="x")
                for k in range(KT):
                    nc.sync.dma_start(out=xsb[:, k, :w],
                                      in_=xT[k * 128:(k + 1) * 128, csl])
                return xsb

            def load_first():
                # W1 first (first chunk's GEMM1 gates the whole pipeline)
                for k in range(KT):
                    nc.sync.dma_start(out=w1sb[:, k, :],
                                      in_=W1[k * 128:(k + 1) * 128, :])
                xsb0 = load_x(*chunks[0])
                for k in range(KT):
                    nc.sync.dma_start(out=wdnsb[:, k, :],
                                      in_=WDN[k * 128:(k + 1) * 128, :])
                nc.sync.dma_start(out=b1sb, in_=B1)
                nc.sync.dma_start(out=prxsb, in_=PRX)
                return xsb0

            def load_rest():
                # needed only by the first stage2, one chunk later
                for k in range(MT):
                    nc.sync.dma_start(out=w2sb[:, k, :],
                                      in_=W2[k * 128:(k + 1) * 128, :])
                nc.sync.dma_start(out=wupsb, in_=WUP)
                nc.sync.dma_start(out=b2sb, in_=B2)

            def stage1(off, w, xsb):
                """GEMM1+GELU, lora down + routing weight."""
                csl = slice(off, off + w)
                hsb = hact.tile([128, MT, CH], wdt, tag="h")
                for m in range(MT):
                    pst = ph.tile([128, CH], F32, tag="ph")
                    for k in range(KT):
                        nc.tensor.matmul(
                            pst[:, :w],
                            w1sb[:, k, m * 128:(m + 1) * 128],
                            xsb[:, k, :w],
                            start=(k == 0), stop=(k == KT - 1))
                    nc.scalar.activation(
                        out=hsb[:, m, :w], in_=pst[:, :w], func=actf,
                        bias=b1sb[:, m:m + 1], scale=1.0)
                psl = ps.tile([ER, CH], F32, tag="pl")
                for k in range(KT):
                    nc.tensor.matmul(
                        psl[:, :w], wdnsb[:, k, :], xsb[:, k, :w],
                        start=(k == 0), stop=(k == KT - 1))
                hl = lora.tile([ER, CH], F32, tag="hl")
                nc.scalar.activation(out=hl[:, :w], in_=psl[:, :w], func=actf)
                hw = lora.tile([ER, CH], wdt, tag="hw")
                nc.vector.tensor_tensor(out=hw[:, :w], in0=hl[:, :w],
                                        in1=prxsb[:, csl],
                                        op=mybir.AluOpType.mult)
                return hsb, hw

            def stage2(off, w, hsb, hw):
                """GEMM2 + accumulated lora up, bias, store."""
                csl = slice(off, off + w)
                osb = outp.tile([128, KT, CH], F32, tag="o")
                for d in range(KT):
                    pso = po.tile([128, CH], F32, tag="po")
                    for k in range(MT):
                        nc.tensor.matmul(
                            pso[:, :w],
                            w2sb[:, k, d * 128:(d + 1) * 128],
                            hsb[:, k, :w],
                            start=(k == 0), stop=False)
                    nc.tensor.matmul(
                        pso[:, :w], wupsb[:, d * 128:(d + 1) * 128],
                        hw[:, :w], start=False, stop=True)
                    nc.vector.tensor_scalar(
                        out=osb[:, d, :w], in0=pso[:, :w],
                        scalar1=b2sb[:, d:d + 1], scalar2=None,
                        op0=mybir.AluOpType.add)
                    nc.sync.dma_start(out=OUT[d * 128:(d + 1) * 128, csl],
                                      in_=osb[:, d, :w])

            def body(_iv=None):
                xsb = load_first()
                # software pipeline: stage2 of chunk j runs one chunk behind
                prev = None
                for ci, (off, w) in enumerate(chunks):
                    if ci + 1 < len(chunks):
                        xsb_next = load_x(*chunks[ci + 1])
                    cur = (off, w) + stage1(off, w, xsb)
                    if ci == 0:
                        load_rest()
                    if prev is not None:
                        stage2(*prev)
                    prev = cur
                    xsb = xsb_next if ci + 1 < len(chunks) else None
                stage2(*prev)

            if nrep == 1:
                body()
            else:
                with tc.For_i(0, nrep, 1,
                              hint_engines=(mybir.EngineType.PE,
                                            mybir.EngineType.Activation,
                                            mybir.EngineType.DVE,
                                            mybir.EngineType.SP)):
                    body()

    nc.compile()
    return nc


# ---------------- host-side helpers ----------------

def shard_inputs(x, topk_probs, topk_idx, w_down, w_up, W1, b1, W2, b2,
                 n_cores=8, mm="bf16", scaling=1.0):
    """Full inputs -> list of per-core in_maps (plus NT per core)."""
    npdt = np.float32 if mm == "fp32r" else ml_dtypes.bfloat16
    x_flat = np.asarray(x, np.float32).reshape(-1, DIM)
    N = x_flat.shape[0]
    assert N % (n_cores * 128) == 0
    NT = N // n_cores

    W1h = np.ascontiguousarray(np.asarray(W1, np.float32)).astype(npdt)
    W2h = np.ascontiguousarray(np.asarray(W2, np.float32)).astype(npdt)
    wdn = np.concatenate([np.asarray(w_down[i], np.float32) for i in range(E)],
                         axis=1).astype(npdt)                       # [DIM, ER]
    wup = (np.concatenate([np.asarray(w_up[i], np.float32) for i in range(E)],
                          axis=0) * scaling).astype(npdt)           # [ER, DIM]
    b1c = np.ascontiguousarray(np.asarray(b1, np.float32).reshape(MT, 128).T)
    b2c = np.ascontiguousarray(np.asarray(b2, np.float32).reshape(KT, 128).T)

    # expanded routing weights: prx[e*R + r, t] = prob of expert e at token t
    idx = np.asarray(topk_idx)
    prb = np.asarray(topk_probs, np.float32)
    probE = np.zeros((E, N), np.float32)
    for i in range(E):
        probE[i] = np.where(idx == i, prb, 0.0).sum(axis=1)
    prx_full = np.repeat(probE, R, axis=0)                          # [ER, N]

    in_maps = []
    for c in range(n_cores):
        sl = slice(c * NT, (c + 1) * NT)
        xTc = np.ascontiguousarray(x_flat[sl].T).astype(npdt)
        prxc = np.ascontiguousarray(prx_full[:, sl]).astype(npdt)
        in_maps.append({
            "xT": xTc, "W1": W1h, "W2": W2h, "wdn": wdn, "wup": wup,
            "b1c": b1c, "b2c": b2c, "prx": prxc,
        })
    return in_maps, NT


def unshard_output(results, x_shape):
    outs = [r["outT"] for r in results]          # each [DIM, NT] f32
    full = np.concatenate(outs, axis=1)          # [DIM, N]
    return np.ascontiguousarray(full.T).reshape(x_shape)


# ---------------- self-contained entry point ----------------

_NC_CACHE = {}


def _get_nc(NT, mm="bf16", nrep=1):
    key = (NT, mm, nrep)
    if key not in _NC_CACHE:
        _NC_CACHE[key] = build_nc(NT, mm=mm, nrep=nrep, num_devices=8,
                                  act="gelu")
    return _NC_CACHE[key]


def kernel(x, gate, topk_probs, topk_idx, w_down, w_up, W1, b1, W2, b2):
    """Full (unsharded) inputs -> full output, 8-core data parallel over
    tokens.  `gate` is unused (the reference never reads it)."""
    from concourse.bass_utils import run_bass_kernel_spmd

    x = np.asarray(x)
    in_maps, NT = shard_inputs(
        x, np.asarray(topk_probs), np.asarray(topk_idx), np.asarray(w_down),
        np.asarray(w_up), np.asarray(W1), np.asarray(b1), np.asarray(W2),
        np.asarray(b2), n_cores=8, mm="bf16", scaling=8.0 / 8.0)
    nc = _get_nc(NT, mm="bf16", nrep=1)
    res = run_bass_kernel_spmd(nc, in_maps, core_ids=list(range(8)))
    return unshard_output(res.results, x.shape).astype(np.float32)


# revision 4
# speedup vs baseline: 1.0788x; 1.0321x over previous
"""ConvNeXt MLP + parallel MoE-LoRA kernel for TRN2 (8-core data parallel).

Per-core computation (tokens sharded across cores, feature-on-partition layout):
  orig = GELU(x @ W1 + b1) @ W2 + b2                       (base MLP)
  h    = GELU(x @ w_down_all)                               (LoRA down, stacked)
  moe  = sum_i (h_i * prob_i) @ w_up_i                      (weighted up-proj)
  out  = orig + moe

All GEMMs keep features on the partition dim, tokens on the free dim; the host
ships x transposed ([DIM, NT] per core) and reassembles the transposed output.
Routing weights are expanded on the host to [ER, NT] (prob of the owning
expert replicated across its R lora dims), so the device does no routing work.

Token chunks are processed in pairs sharing each stationary weight load
(LDW w; MM chunkA; MM chunkB), and a post-compile pass deletes the redundant
second Ldweights of every pair — on HW the weight (re)load serializes with the
matmul stream, so halving the Ldweights count buys real PE issue time.
"""

import numpy as np
import ml_dtypes

import concourse.bass as bass
import concourse.mybir as mybir
import concourse.tile as tile
from concourse import bacc

F32 = mybir.dt.float32
F32R = mybir.dt.float32r
BF16 = mybir.dt.bfloat16

DIM, HID, E, R = 384, 1536, 3, 8
KT = DIM // 128    # 3  k-tiles of x / W1 rows
MT = HID // 128    # 12 hid tiles
ER = E * R         # 24 stacked lora dims
CH = 512           # token chunk (free dim per matmul)


def _ap_sig(arg):
    """Stable signature for an instruction Argument (weights AP)."""
    try:
        return repr(arg)
    except Exception:
        return None


def dedup_ldweights(nc):
    """Delete InstLdweights that reload the identical weights AP while only
    Matmult instructions ran on PE since the previous load.  Safe because the
    paired InstMatmult keeps the weights AP in its `ins` (dependency graph
    unchanged) and a dropped Ldweights carries no semaphore waits/updates."""
    removed = 0
    for blk in nc.m.functions[0].blocks:
        last_sig = None
        kept = []
        for inst in blk.instructions:
            nm = type(inst).__name__
            if nm == "InstLdweights":
                si = inst.sync_info
                clean = si is None or (not si.on_wait and not si.on_update)
                sig = _ap_sig(inst.ins[0])
                if clean and sig is not None and sig == last_sig:
                    removed += 1
                    continue
                last_sig = sig
            elif nm == "InstMatmult":
                if getattr(inst, "is_transpose", False):
                    last_sig = None
            else:
                eng = getattr(inst, "engine", None)
                if eng == mybir.EngineType.PE:
                    last_sig = None
            kept.append(inst)
        blk.instructions[:] = kept
    return removed


def build_nc(NT, mm="bf16", nrep=1, num_devices=8, act="gelu", dedup=True):
    """Build the bass program for one core's shard of NT tokens."""
    assert NT % 128 == 0
    wdt = F32R if mm == "fp32r" else BF16
    actf = (mybir.ActivationFunctionType.Gelu if act == "gelu"
            else mybir.ActivationFunctionType.Identity)

    nc = bacc.Bacc("TRN2", target_bir_lowering=False, debug=False,
                   num_devices=num_devices)

    xT = nc.dram_tensor("xT", [DIM, NT], wdt, kind="ExternalInput").ap()
    W1 = nc.dram_tensor("W1", [DIM, HID], wdt, kind="ExternalInput").ap()
    W2 = nc.dram_tensor("W2", [HID, DIM], wdt, kind="ExternalInput").ap()
    WDN = nc.dram_tensor("wdn", [DIM, ER], wdt, kind="ExternalInput").ap()
    WUP = nc.dram_tensor("wup", [ER, DIM], wdt, kind="ExternalInput").ap()
    B1 = nc.dram_tensor("b1c", [128, MT], F32, kind="ExternalInput").ap()
    B2 = nc.dram_tensor("b2c", [128, KT], F32, kind="ExternalInput").ap()
    PRX = nc.dram_tensor("prx", [ER, NT], wdt, kind="ExternalInput").ap()
    OUT = nc.dram_tensor("outT", [DIM, NT], F32, kind="ExternalOutput").ap()

    # token chunks, processed in pairs sharing stationary weights
    chunks = []
    off = 0
    while off < NT:
        w = min(CH, NT - off)
        chunks.append((off, w))
        off += w
    groups = [tuple(chunks[i:i + 2]) for i in range(0, len(chunks), 2)]

    with tile.TileContext(nc) as tc:
        with (
            tc.tile_pool(name="const", bufs=1) as const,
            tc.tile_pool(name="xin", bufs=4) as xin,
            tc.tile_pool(name="hact", bufs=4) as hact,
            tc.tile_pool(name="lora", bufs=2) as lora,
            tc.tile_pool(name="lorw", bufs=4) as lorw,
            tc.tile_pool(name="outp", bufs=3) as outp,
            tc.tile_pool(name="ph", bufs=4, space="PSUM") as ph,
            tc.tile_pool(name="po", bufs=4, space="PSUM") as po,
        ):
            w1sb = const.tile([128, KT, HID], wdt)
            w2sb = const.tile([128, MT, DIM], wdt)
            wdnsb = const.tile([128, KT, ER], wdt)
            wupsb = const.tile([ER, DIM], wdt)
            b1sb = const.tile([128, MT], F32)
            b2sb = const.tile([128, KT], F32)
            prxsb = const.tile([ER, NT], wdt)

            def load_x(off, w):
                csl = slice(off, off + w)
                xsb = xin.tile([128, KT, CH], wdt, tag="x")
                for k in range(KT):
                    nc.sync.dma_start(out=xsb[:, k, :w],
                                      in_=xT[k * 128:(k + 1) * 128, csl])
                return xsb

            def load_first():
                # W1 first (first group's GEMM1 gates the whole pipeline)
                for k in range(KT):
                    nc.sync.dma_start(out=w1sb[:, k, :],
                                      in_=W1[k * 128:(k + 1) * 128, :])
                xs0 = [load_x(off, w) for (off, w) in groups[0]]
                for k in range(KT):
                    nc.sync.dma_start(out=wdnsb[:, k, :],
                                      in_=WDN[k * 128:(k + 1) * 128, :])
                nc.sync.dma_start(out=b1sb, in_=B1)
                nc.sync.dma_start(out=prxsb, in_=PRX)
                return xs0

            def load_rest():
                # needed only by the first stage2, one group later
                for k in range(MT):
                    nc.sync.dma_start(out=w2sb[:, k, :],
                                      in_=W2[k * 128:(k + 1) * 128, :])
                nc.sync.dma_start(out=wupsb, in_=WUP)
                nc.sync.dma_start(out=b2sb, in_=B2)

            def stage1_group(grp, xsbs):
                """GEMM1 + GELU + lora down for a pair of chunks; each
                stationary weight streams all chunks of the group."""
                n = len(grp)
                hsbs = [hact.tile([128, MT, CH], wdt, tag="h", name=f"hsb{ci}")
                        for ci in range(len(grp))]
                for m in range(MT):
                    psts = [ph.tile([128, CH], F32, tag="ph", name=f"pst{ci}")
                            for ci in range(len(grp))]
                    for k in range(KT):
                        for ci, (off, w) in enumerate(grp):
                            nc.tensor.matmul(
                                psts[ci][:, :w],
                                w1sb[:, k, m * 128:(m + 1) * 128],
                                xsbs[ci][:, k, :w],
                                start=(k == 0), stop=(k == KT - 1))
                    for ci, (off, w) in enumerate(grp):
                        nc.scalar.activation(
                            out=hsbs[ci][:, m, :w], in_=psts[ci][:, :w],
                            func=actf, bias=b1sb[:, m:m + 1], scale=1.0)
                # lora down as a 13th (24-wide) output tile
                psls = [ph.tile([128, CH], F32, tag="ph", name=f"psl{ci}")
                        for ci in range(len(grp))]
                for k in range(KT):
                    for ci, (off, w) in enumerate(grp):
                        nc.tensor.matmul(
                            psls[ci][:ER, :w], wdnsb[:, k, :],
                            xsbs[ci][:, k, :w],
                            start=(k == 0), stop=(k == KT - 1))
                hws = []
                for ci, (off, w) in enumerate(grp):
                    csl = slice(off, off + w)
                    hl = lora.tile([ER, CH], F32, tag="hl")
                    nc.scalar.activation(out=hl[:, :w], in_=psls[ci][:ER, :w],
                                         func=actf)
                    hw = lorw.tile([ER, CH], wdt, tag="hw")
                    nc.vector.tensor_tensor(out=hw[:, :w], in0=hl[:, :w],
                                            in1=prxsb[:, csl],
                                            op=mybir.AluOpType.mult)
                    hws.append(hw)
                return hsbs, hws

            def stage2_group(grp, hsbs, hws):
                """GEMM2 + accumulated lora up, bias, store."""
                for d in range(KT):
                    psos = [po.tile([128, CH], F32, tag="po", name=f"pso{ci}")
                            for ci in range(len(grp))]
                    for k in range(MT):
                        for ci, (off, w) in enumerate(grp):
                            nc.tensor.matmul(
                                psos[ci][:, :w],
                                w2sb[:, k, d * 128:(d + 1) * 128],
                                hsbs[ci][:, k, :w],
                                start=(k == 0), stop=False)
                    for ci, (off, w) in enumerate(grp):
                        nc.tensor.matmul(
                            psos[ci][:, :w], wupsb[:, d * 128:(d + 1) * 128],
                            hws[ci][:, :w], start=False, stop=True)
                    for ci, (off, w) in enumerate(grp):
                        csl = slice(off, off + w)
                        osb = outp.tile([128, CH], F32, tag="o")
                        nc.vector.tensor_scalar(
                            out=osb[:, :w], in0=psos[ci][:, :w],
                            scalar1=b2sb[:, d:d + 1], scalar2=None,
                            op0=mybir.AluOpType.add)
                        nc.sync.dma_start(out=OUT[d * 128:(d + 1) * 128, csl],
                                          in_=osb[:, :w])

            def body(_iv=None):
                xs = load_first()
                prev = None
                for gi, grp in enumerate(groups):
                    if gi + 1 < len(groups):
                        xs_next = [load_x(off, w) for (off, w) in groups[gi + 1]]
                    cur = (grp,) + stage1_group(grp, xs)
                    if gi == 0:
                        load_rest()
                    if prev is not None:
                        stage2_group(*prev)
                    prev = cur
                    xs = xs_next if gi + 1 < len(groups) else None
                stage2_group(*prev)

            if nrep == 1:
                body()
            else:
                with tc.For_i(0, nrep, 1,
                              hint_engines=(mybir.EngineType.PE,
                                            mybir.EngineType.Activation,
                                            mybir.EngineType.DVE,
                                            mybir.EngineType.SP)):
                    body()

    nc.compile()
    if dedup:
        dedup_ldweights(nc)
    return nc


# ---------------- host-side helpers ----------------

def shard_inputs(x, topk_probs, topk_idx, w_down, w_up, W1, b1, W2, b2,
                 n_cores=8, mm="bf16", scaling=1.0):
    """Full inputs -> list of per-core in_maps (plus NT per core)."""
    npdt = np.float32 if mm == "fp32r" else ml_dtypes.bfloat16
    x_flat = np.asarray(x, np.float32).reshape(-1, DIM)
    N = x_flat.shape[0]
    assert N % (n_cores * 128) == 0
    NT = N // n_cores

    W1h = np.ascontiguousarray(np.asarray(W1, np.float32)).astype(npdt)
    W2h = np.ascontiguousarray(np.asarray(W2, np.float32)).astype(npdt)
    wdn = np.concatenate([np.asarray(w_down[i], np.float32) for i in range(E)],
                         axis=1).astype(npdt)                       # [DIM, ER]
    wup = (np.concatenate([np.asarray(w_up[i], np.float32) for i in range(E)],
                          axis=0) * scaling).astype(npdt)           # [ER, DIM]
    b1c = np.ascontiguousarray(np.asarray(b1, np.float32).reshape(MT, 128).T)
    b2c = np.ascontiguousarray(np.asarray(b2, np.float32).reshape(KT, 128).T)

    # expanded routing weights: prx[e*R + r, t] = prob of expert e at token t
    idx = np.asarray(topk_idx)
    prb = np.asarray(topk_probs, np.float32)
    probE = np.zeros((E, N), np.float32)
    for i in range(E):
        probE[i] = np.where(idx == i, prb, 0.0).sum(axis=1)
    prx_full = np.repeat(probE, R, axis=0)                          # [ER, N]

    in_maps = []
    for c in range(n_cores):
        sl = slice(c * NT, (c + 1) * NT)
        xTc = np.ascontiguousarray(x_flat[sl].T).astype(npdt)
        prxc = np.ascontiguousarray(prx_full[:, sl]).astype(npdt)
        in_maps.append({
            "xT": xTc, "W1": W1h, "W2": W2h, "wdn": wdn, "wup": wup,
            "b1c": b1c, "b2c": b2c, "prx": prxc,
        })
    return in_maps, NT


def unshard_output(results, x_shape):
    outs = [r["outT"] for r in results]          # each [DIM, NT] f32
    full = np.concatenate(outs, axis=1)          # [DIM, N]
    return np.ascontiguousarray(full.T).reshape(x_shape)


# ---------------- self-contained entry point ----------------

_NC_CACHE = {}


def _get_nc(NT, mm="bf16", nrep=1):
    key = (NT, mm, nrep)
    if key not in _NC_CACHE:
        _NC_CACHE[key] = build_nc(NT, mm=mm, nrep=nrep, num_devices=8,
                                  act="gelu")
    return _NC_CACHE[key]


def kernel(x, gate, topk_probs, topk_idx, w_down, w_up, W1, b1, W2, b2):
    """Full (unsharded) inputs -> full output, 8-core data parallel over
    tokens.  `gate` is unused (the reference never reads it)."""
    from concourse.bass_utils import run_bass_kernel_spmd

    x = np.asarray(x)
    in_maps, NT = shard_inputs(
        x, np.asarray(topk_probs), np.asarray(topk_idx), np.asarray(w_down),
        np.asarray(w_up), np.asarray(W1), np.asarray(b1), np.asarray(W2),
        np.asarray(b2), n_cores=8, mm="bf16", scaling=8.0 / 8.0)
    nc = _get_nc(NT, mm="bf16", nrep=1)
    res = run_bass_kernel_spmd(nc, in_maps, core_ids=list(range(8)))
    return unshard_output(res.results, x.shape).astype(np.float32)
